# revision 13
# baseline (speedup 1.0000x reference)
"""LoRA linear y = x @ (B@A).T computed low-rank: y = (x @ A.T) @ B.T.

Sharding: data-parallel over tokens (B*S = 16384) across 8 NeuronCores,
2048 tokens/core; lora_A / lora_B replicated (tiny).

Per-core pipeline (supertiles of 256 tokens):
  DMA x tile [128, 4096] -> PE transpose (fp32, 128x128 blocks via identity)
  -> xT in SBUF -> mm1 tT[16, 256] = sum_c AT_c.T @ xT_c  (fp32r, K=128 x32)
  -> mm2 y[128, 512] = tT_h.T @ BT_nb                     (fp32r, K=16)
  -> DVE copies PSUM->SBUF -> DMA out.
"""

import os
import numpy as np

import concourse.bass as bass
import concourse.mybir as mybir
from concourse.tile import TileContext
from concourse.bass_utils import run_bass_kernel_spmd
from concourse.masks import make_identity

N_CORES = 8
B, S, D_IN, D_OUT, R = 4, 4096, 4096, 4096, 16
TOK = B * S
TPC = TOK // N_CORES  # tokens per core: 2048
ST = 256              # supertile tokens
N_ST = TPC // ST      # 8
F32 = mybir.dt.float32
F32R = mybir.dt.float32r

_DT = {"f32": F32, "f32r": F32R}
MM1_DT = _DT[os.environ.get("MM1_DT", "f32r")]
MM2_DT = _DT[os.environ.get("MM2_DT", "f32r")]
TR_R = os.environ.get("TR_DT", "f32r") == "f32r"  # fp32r transposes
ACT_COPIES = int(os.environ.get("ACT_COPIES", "2"))  # y-copies per h on ACT


def _split_drain_waits(nc):
    """This walrus build rejects instructions carrying >1 sem wait; hoist
    extra waits onto preceding single-wait NoOps on the same engine."""
    f = nc.m.functions[0]

    def fix_bb(bb):
        insts = getattr(bb, "instructions", None)
        if insts:
            new = []
            for inst in insts:
                si = inst.sync_info
                if si is not None and si.on_wait is not None and len(si.on_wait) > 1:
                    waits = list(si.on_wait)
                    for w in waits[:-1]:
                        d = mybir.InstNoOp(
                            name=nc.get_next_instruction_name(), ins=[], outs=[]
                        )
                        d.engine = inst.engine
                        d.sync_info = mybir.SyncInfo(on_wait=[w], on_update=[])
                        new.append(d)
                    si.on_wait = [waits[-1]]
                    inst.sync_info = si
                new.append(inst)
            bb.instructions[:] = new
        for sub in getattr(bb, "blocks", []) or []:
            fix_bb(sub)

    for blk in f.blocks:
        fix_bb(blk)


def _build():
    nc = bass.Bass("TRN2", target_bir_lowering=False, debug=False, num_devices=N_CORES)
    xs = nc.declare_dram_parameter("xs", [TPC, D_IN], F32R if TR_R else F32, isOutput=False)
    atp = nc.declare_dram_parameter("atp", [128, 32 * R], F32, isOutput=False)
    bt = nc.declare_dram_parameter("bt", [R, D_OUT], F32, isOutput=False)
    ys = nc.declare_dram_parameter("ys", [TPC, D_OUT], F32, isOutput=True)

    with TileContext(nc) as tc:
        with (
            tc.tile_pool(name="const", bufs=1) as cpool,
            tc.tile_pool(name="x", bufs=3) as xpool,
            tc.tile_pool(name="xt", bufs=2) as xtpool,
            tc.tile_pool(name="y", bufs=3) as ypool,
            tc.tile_pool(name="t", bufs=2) as tpool,
            tc.tile_pool(name="tp_ps", bufs=3, space="PSUM") as tppsum,
            tc.tile_pool(name="t_ps", bufs=2, space="PSUM") as tpsum,
            tc.tile_pool(name="y_ps", bufs=3, space="PSUM") as ypsum,
        ):
            if MM1_DT == F32:
                at_sb = cpool.tile([128, 32 * R], F32)
                nc.sync.dma_start(out=at_sb[:], in_=atp[:])
            else:
                at_raw = cpool.tile([128, 32 * R], F32)
                nc.sync.dma_start(out=at_raw[:], in_=atp[:])
                at_sb = cpool.tile([128, 32 * R], MM1_DT)
                nc.vector.tensor_copy(out=at_sb[:], in_=at_raw[:])
            if MM2_DT == F32:
                bt_sb = cpool.tile([R, D_OUT], F32)
                nc.sync.dma_start(out=bt_sb[:], in_=bt[:])
            else:
                bt_raw = cpool.tile([R, D_OUT], F32)
                nc.sync.dma_start(out=bt_raw[:], in_=bt[:])
                bt_sb = cpool.tile([R, D_OUT], MM2_DT)
                nc.vector.tensor_copy(out=bt_sb[:], in_=bt_raw[:])
            ident = cpool.tile([128, 128], F32)
            make_identity(nc, ident[:])
            if TR_R:
                ident_r = cpool.tile([128, 128], F32R)
                nc.vector.tensor_copy(out=ident_r[:], in_=ident[:])
            else:
                ident_r = ident

            for st in range(N_ST):
                base = st * ST
                xt = xtpool.tile([128, 2, 32, 128], MM1_DT)
                for h in range(2):
                    xh = xpool.tile([128, D_IN], F32R if TR_R else F32)
                    for q in range(2):
                        nc.sync.dma_start(
                            out=xh[:, q * 2048 : (q + 1) * 2048],
                            in_=xs[
                                base + h * 128 : base + (h + 1) * 128,
                                q * 2048 : (q + 1) * 2048,
                            ],
                        )
                    for cg in range(8):
                        pt = tppsum.tile([128, 4, 128], F32R if TR_R else F32)
                        for j in range(4):
                            c = cg * 4 + j
                            if TR_R:
                                nc.tensor.matmul(
                                    pt[:, j, :],
                                    xh[:, c * 128 : (c + 1) * 128],
                                    ident_r[:],
                                    is_transpose=True,
                                )
                            else:
                                nc.tensor.transpose(
                                    pt[:, j, :], xh[:, c * 128 : (c + 1) * 128], ident[:]
                                )
                        nc.vector.tensor_copy(
                            out=xt[:, h, cg * 4 : cg * 4 + 4, :],
                            in_=pt[:],
                        )
                # mm1: tT[16, ST] accumulated over 32 d-chunks
                tps = tpsum.tile([R, ST], F32)
                for c in range(32):
                    nc.tensor.matmul(
                        tps[:],
                        at_sb[:, c * R : (c + 1) * R],
                        xt[:, :, c, :],
                        start=(c == 0),
                        stop=(c == 31),
                    )
                t_sb = tpool.tile([R, ST], MM2_DT)
                nc.vector.tensor_copy(out=t_sb[:], in_=tps[:])
                # mm2: y[128, 512] blocks
                for h in range(2):
                    y_sb = ypool.tile([128, D_OUT], F32)
                    for nb in range(8):
                        yps = ypsum.tile([128, 512], F32)
                        nc.tensor.matmul(
                            yps[:],
                            t_sb[:, h * 128 : (h + 1) * 128],
                            bt_sb[:, nb * 512 : (nb + 1) * 512],
                            start=True,
                            stop=True,
                        )
                        if nb >= 8 - ACT_COPIES:
                            nc.scalar.activation(
                                out=y_sb[:, nb * 512 : (nb + 1) * 512],
                                in_=yps[:],
                                func=mybir.ActivationFunctionType.Identity,
                            )
                        else:
                            nc.vector.tensor_copy(
                                out=y_sb[:, nb * 512 : (nb + 1) * 512], in_=yps[:]
                            )
                    for q in range(2):
                        nc.scalar.dma_start(
                            out=ys[
                                base + h * 128 : base + (h + 1) * 128,
                                q * 2048 : (q + 1) * 2048,
                            ],
                            in_=y_sb[:, q * 2048 : (q + 1) * 2048],
                        )

    _split_drain_waits(nc)
    return nc


_NC = None


def _get_nc():
    global _NC
    if _NC is None:
        _NC = _build()
    return _NC


def _prep_inputs(x, lora_A, lora_B):
    x_flat = np.ascontiguousarray(np.asarray(x, dtype=np.float32).reshape(TOK, D_IN))
    A = np.asarray(lora_A, dtype=np.float32)
    Bm = np.asarray(lora_B, dtype=np.float32)
    # atp[p, c*R + r] = A[r, c*128 + p]
    atp = np.ascontiguousarray(
        A.T.reshape(32, 128, R).transpose(1, 0, 2).reshape(128, 32 * R)
    )
    btv = np.ascontiguousarray(Bm.T)
    return x_flat, atp, btv


def kernel(x, lora_A, lora_B, _trace=False, _trace_kwargs=None):
    nc = _get_nc()
    x_flat, atp, btv = _prep_inputs(x, lora_A, lora_B)
    in_maps = [
        {"xs": x_flat[i * TPC : (i + 1) * TPC], "atp": atp, "bt": btv}
        for i in range(N_CORES)
    ]
    res = run_bass_kernel_spmd(
        nc, in_maps, list(range(N_CORES)), trace=_trace, **(_trace_kwargs or {})
    )
    y = np.concatenate([res.results[i]["ys"] for i in range(N_CORES)], axis=0)
    out = y.reshape(B, S, D_OUT)
    if _trace:
        return out, res
    return out


# revision 14
# speedup vs baseline: 1.0375x; 1.0375x over previous
"""LoRA linear y = x @ (B@A).T computed low-rank: y = (x @ A.T) @ B.T.

Sharding: data-parallel over tokens (B*S = 16384) across 8 NeuronCores,
2048 tokens/core; lora_A / lora_B replicated (tiny).

Per-core pipeline (supertiles of 256 tokens):
  DMA x tile [128, 4096] -> PE transpose (fp32, 128x128 blocks via identity)
  -> xT in SBUF -> mm1 tT[16, 256] = sum_c AT_c.T @ xT_c  (fp32r, K=128 x32)
  -> mm2 y[128, 512] = tT_h.T @ BT_nb                     (fp32r, K=16)
  -> DVE copies PSUM->SBUF -> DMA out.
"""

import os
import numpy as np

import concourse.bass as bass
import concourse.mybir as mybir
from concourse.tile import TileContext
from concourse.bass_utils import run_bass_kernel_spmd
from concourse.masks import make_identity

N_CORES = 8
B, S, D_IN, D_OUT, R = 4, 4096, 4096, 4096, 16
TOK = B * S
TPC = TOK // N_CORES  # tokens per core: 2048
ST = 256              # supertile tokens
N_ST = TPC // ST      # 8
F32 = mybir.dt.float32
F32R = mybir.dt.float32r

_DT = {"f32": F32, "f32r": F32R}
MM1_DT = _DT[os.environ.get("MM1_DT", "f32r")]
MM2_DT = _DT[os.environ.get("MM2_DT", "f32r")]
TR_R = os.environ.get("TR_DT", "f32r") == "f32r"  # fp32r transposes
ACT_COPIES = int(os.environ.get("ACT_COPIES", "2"))  # y-copies per h on ACT


def _split_drain_waits(nc):
    """This walrus build rejects instructions carrying >1 sem wait; hoist
    extra waits onto preceding single-wait NoOps on the same engine."""
    f = nc.m.functions[0]

    def fix_bb(bb):
        insts = getattr(bb, "instructions", None)
        if insts:
            new = []
            for inst in insts:
                si = inst.sync_info
                if si is not None and si.on_wait is not None and len(si.on_wait) > 1:
                    waits = list(si.on_wait)
                    for w in waits[:-1]:
                        d = mybir.InstNoOp(
                            name=nc.get_next_instruction_name(), ins=[], outs=[]
                        )
                        d.engine = inst.engine
                        d.sync_info = mybir.SyncInfo(on_wait=[w], on_update=[])
                        new.append(d)
                    si.on_wait = [waits[-1]]
                    inst.sync_info = si
                new.append(inst)
            bb.instructions[:] = new
        for sub in getattr(bb, "blocks", []) or []:
            fix_bb(sub)

    for blk in f.blocks:
        fix_bb(blk)


def _build():
    nc = bass.Bass("TRN2", target_bir_lowering=False, debug=False, num_devices=N_CORES)
    xs = nc.declare_dram_parameter("xs", [TPC, D_IN], F32R if TR_R else F32, isOutput=False)
    atp = nc.declare_dram_parameter("atp", [128, 32 * R], F32, isOutput=False)
    bt = nc.declare_dram_parameter("bt", [R, D_OUT], F32, isOutput=False)
    ys = nc.declare_dram_parameter("ys", [TPC, D_OUT], F32, isOutput=True)

    with TileContext(nc) as tc:
        with (
            tc.tile_pool(name="const", bufs=1) as cpool,
            tc.tile_pool(name="x", bufs=3) as xpool,
            tc.tile_pool(name="xt", bufs=2) as xtpool,
            tc.tile_pool(name="y", bufs=3) as ypool,
            tc.tile_pool(name="t", bufs=2) as tpool,
            tc.tile_pool(name="tp_ps", bufs=3, space="PSUM") as tppsum,
            tc.tile_pool(name="t_ps", bufs=2, space="PSUM") as tpsum,
            tc.tile_pool(name="y_ps", bufs=3, space="PSUM") as ypsum,
        ):
            if MM1_DT == F32:
                at_sb = cpool.tile([128, 32 * R], F32)
                nc.scalar.dma_start(out=at_sb[:], in_=atp[:])
            else:
                at_raw = cpool.tile([128, 32 * R], F32)
                nc.scalar.dma_start(out=at_raw[:], in_=atp[:])
                at_sb = cpool.tile([128, 32 * R], MM1_DT)
                nc.vector.tensor_copy(out=at_sb[:], in_=at_raw[:])
            if MM2_DT == F32:
                bt_sb = cpool.tile([R, D_OUT], F32)
                nc.scalar.dma_start(out=bt_sb[:], in_=bt[:])
            else:
                bt_raw = cpool.tile([R, D_OUT], F32)
                nc.scalar.dma_start(out=bt_raw[:], in_=bt[:])
                bt_sb = cpool.tile([R, D_OUT], MM2_DT)
                nc.vector.tensor_copy(out=bt_sb[:], in_=bt_raw[:])
            ident = cpool.tile([128, 128], F32)
            make_identity(nc, ident[:])
            if TR_R:
                ident_r = cpool.tile([128, 128], F32R)
                nc.vector.tensor_copy(out=ident_r[:], in_=ident[:])
            else:
                ident_r = ident

            for st in range(N_ST):
                base = st * ST
                xt = xtpool.tile([128, 2, 32, 128], MM1_DT)
                for h in range(2):
                    xh = xpool.tile([128, D_IN], F32R if TR_R else F32)
                    for q in range(2):
                        nc.sync.dma_start(
                            out=xh[:, q * 2048 : (q + 1) * 2048],
                            in_=xs[
                                base + h * 128 : base + (h + 1) * 128,
                                q * 2048 : (q + 1) * 2048,
                            ],
                        )
                    for cg in range(8):
                        pt = tppsum.tile([128, 4, 128], F32R if TR_R else F32)
                        for j in range(4):
                            c = cg * 4 + j
                            if TR_R:
                                nc.tensor.matmul(
                                    pt[:, j, :],
                                    xh[:, c * 128 : (c + 1) * 128],
                                    ident_r[:],
                                    is_transpose=True,
                                )
                            else:
                                nc.tensor.transpose(
                                    pt[:, j, :], xh[:, c * 128 : (c + 1) * 128], ident[:]
                                )
                        nc.vector.tensor_copy(
                            out=xt[:, h, cg * 4 : cg * 4 + 4, :],
                            in_=pt[:],
                        )
                # mm1: tT[16, ST] accumulated over 32 d-chunks
                tps = tpsum.tile([R, ST], F32)
                for c in range(32):
                    nc.tensor.matmul(
                        tps[:],
                        at_sb[:, c * R : (c + 1) * R],
                        xt[:, :, c, :],
                        start=(c == 0),
                        stop=(c == 31),
                    )
                t_sb = tpool.tile([R, ST], MM2_DT)
                nc.vector.tensor_copy(out=t_sb[:], in_=tps[:])
                # mm2: y[128, 512] blocks
                for h in range(2):
                    y_sb = ypool.tile([128, D_OUT], F32)
                    for nb in range(8):
                        yps = ypsum.tile([128, 512], F32)
                        nc.tensor.matmul(
                            yps[:],
                            t_sb[:, h * 128 : (h + 1) * 128],
                            bt_sb[:, nb * 512 : (nb + 1) * 512],
                            start=True,
                            stop=True,
                        )
                        if nb >= 8 - ACT_COPIES:
                            nc.scalar.activation(
                                out=y_sb[:, nb * 512 : (nb + 1) * 512],
                                in_=yps[:],
                                func=mybir.ActivationFunctionType.Identity,
                            )
                        else:
                            nc.vector.tensor_copy(
                                out=y_sb[:, nb * 512 : (nb + 1) * 512], in_=yps[:]
                            )
                    for q in range(2):
                        nc.scalar.dma_start(
                            out=ys[
                                base + h * 128 : base + (h + 1) * 128,
                                q * 2048 : (q + 1) * 2048,
                            ],
                            in_=y_sb[:, q * 2048 : (q + 1) * 2048],
                        )

    _split_drain_waits(nc)
    return nc


_NC = None


def _get_nc():
    global _NC
    if _NC is None:
        _NC = _build()
    return _NC


def _prep_inputs(x, lora_A, lora_B):
    x_flat = np.ascontiguousarray(np.asarray(x, dtype=np.float32).reshape(TOK, D_IN))
    A = np.asarray(lora_A, dtype=np.float32)
    Bm = np.asarray(lora_B, dtype=np.float32)
    # atp[p, c*R + r] = A[r, c*128 + p]
    atp = np.ascontiguousarray(
        A.T.reshape(32, 128, R).transpose(1, 0, 2).reshape(128, 32 * R)
    )
    btv = np.ascontiguousarray(Bm.T)
    return x_flat, atp, btv


def kernel(x, lora_A, lora_B, _trace=False, _trace_kwargs=None):
    nc = _get_nc()
    x_flat, atp, btv = _prep_inputs(x, lora_A, lora_B)
    in_maps = [
        {"xs": x_flat[i * TPC : (i + 1) * TPC], "atp": atp, "bt": btv}
        for i in range(N_CORES)
    ]
    res = run_bass_kernel_spmd(
        nc, in_maps, list(range(N_CORES)), trace=_trace, **(_trace_kwargs or {})
    )
    y = np.concatenate([res.results[i]["ys"] for i in range(N_CORES)], axis=0)
    out = y.reshape(B, S, D_OUT)
    if _trace:
        return out, res
    return out
